# revision 16
# baseline (speedup 1.0000x reference)
"""Trainium2 Bass kernel for nn_ConvolutionalModel_44555990729204.

Math (from the reference):
    win[i,j,:]  = x windows of 4x4 (stride 4), flattened k2 = 4r+c
    rec  = relu(win @ (We@Wr) + (be@Wr + br))          # We@Wr folded: rank-16
    attn = relu(win @ Wa + ba)
    out  = x + (rec @ Ws + bs) * attn   (scattered back to windows)

Strategy: the window gather/scatter is a pure layout permutation, so it is
done host-side while sharding (in_maps construction), pre-cast to f16 —
halving HBM traffic and removing all on-device transposes/shuffles. The
device does all math in window layout:

  win8 [128 = 8 groups x 16 k2, f = windows] f16 per [128,1024] tile
  - pattn16 = Wa16^T win8        (PE, attn pre-act replicated x16 in-matmul)
  - attn16  = relu(pattn16 + ba) (DVE stt, PSUM->SBUF f16)
  - prec_q  = Wcomb2^T win8[q]   (PE, 4 row-tiled concurrent matmuls)
  - rec_q   = relu(prec_q+bcomb) (ACT/DVE split, PSUM->SBUF f16)
  - prep    = Ws2^T rec_q        (PE, 4 col-tiled matmuls)
  - updwin  = (prep+bs)*attn16   (DVE stt, PSUM x SBUF -> f16)
  - outwin  = updwin + win8      (GPSIMD add)
  - DMA out f16; host casts f32 + inverse window scatter.

Per-core: 2 images = 2048 rows = 16 tiles of [128, 1024].
"""

import sys

sys.path.insert(0, "/opt/trn_rl_repo")

import numpy as np

import concourse.bacc as bacc
import concourse.bass as bass
import concourse.mybir as mybir
from concourse import tile
from concourse.alu_op_type import AluOpType
from concourse.bass_utils import run_bass_kernel_spmd

F32 = mybir.dt.float32
F16 = mybir.dt.float16
F8 = mybir.dt.float8e4
RELU = mybir.ActivationFunctionType.Relu

N_CORES = 8
B, H, W = 16, 1024, 1024
BPC = B // N_CORES          # images per core
ROWS = BPC * H              # 2048 rows per core
NT = ROWS // 128            # 16 tiles per core
FH = 512                    # psum bank width in f32

# wconst column layout (f16)
WCOMB2_C = slice(0, 128)    # [32, 128] replicated x4 on partitions
WA16_C = slice(128, 256)    # [128, 128] block-diag Wa replicated
WS2_C = slice(256, 288)     # [128, 32]
WCONST_COLS = 288
# wb column layout (f32)
BCOMB2_C = slice(0, 1)      # [128, 1] bcomb tiled x2
BS2_C = slice(1, 2)         # [128, 1] bs tiled x8
BA_C = slice(2, 3)          # [128, 1] ba scalar bcast
ZERO_C = slice(3, 4)        # [128, 1] zeros
WB_COLS = 8


def _build_wconst(Wa, ba, We, be, Wr, br, Ws, bs):
    Wcomb = We @ Wr                       # [16, 64]
    bcomb = be @ Wr + br                  # [64]

    wconst = np.zeros((128, WCONST_COLS), dtype=np.float32)
    w2 = np.zeros((32, 128), dtype=np.float32)
    w2[0:16, 0:64] = Wcomb
    w2[16:32, 64:128] = Wcomb
    wconst[:, WCOMB2_C] = np.tile(w2, (4, 1))
    wa16 = np.zeros((128, 128), dtype=np.float32)
    for g in range(8):
        wa16[16 * g:16 * g + 16, 16 * g:16 * g + 16] = np.tile(
            Wa[:, 0:1], (1, 16))
    wconst[:, WA16_C] = wa16
    ws2 = np.zeros((128, 32), dtype=np.float32)
    ws2[0:64, 0:16] = Ws
    ws2[64:128, 16:32] = Ws
    wconst[:, WS2_C] = ws2

    # DoubleRow m2 weights (fp8): two M=128 sets (one per q-pair qq), the
    # other half zero so both accumulate into one full-width PSUM bank
    # (DoubleRow rejects non-zero dst partition offsets).
    # lhs_qq[p=64s+d, k, m=64qq+32k+16s+k2] = Ws[d, k2],
    # stored wdr[p, 256qq + 128k + m].
    wdr = np.zeros((128, 512), dtype=np.float32)
    for qq in range(2):
        for k in range(2):
            for s in range(2):
                for k2 in range(16):
                    col = 256 * qq + 128 * k + 64 * qq + 32 * k + 16 * s + k2
                    wdr[64 * s:64 * s + 64, col] = Ws[:, k2]

    wb = np.zeros((128, WB_COLS), dtype=np.float32)
    wb[:, BCOMB2_C] = np.tile(bcomb, 2)[:, None]
    wb[:, BS2_C] = np.tile(bs, 8)[:, None]
    wb[:, BA_C] = float(ba[0])
    import ml_dtypes
    return (wconst.astype(np.float16), wdr.astype(ml_dtypes.float8_e4m3),
            wb)


def _build_nc():
    nc = bacc.Bacc()
    xw = nc.dram_tensor("xw", [ROWS, W], F16, kind="ExternalInput")
    wc = nc.dram_tensor("wc", [128, WCONST_COLS], F16, kind="ExternalInput")
    wb = nc.dram_tensor("wb", [128, WB_COLS], F32, kind="ExternalInput")
    yw = nc.dram_tensor("yw", [ROWS, W], F16, kind="ExternalOutput")

    with tile.TileContext(nc) as tc:
        with (
            tc.tile_pool(name="const", bufs=1) as cpool,
            tc.tile_pool(name="io", bufs=4) as iopool,
            tc.tile_pool(name="attn", bufs=2) as apool,
            tc.tile_pool(name="rec", bufs=4) as recpool,
            tc.tile_pool(name="upd", bufs=2) as upool,
            tc.tile_pool(name="out", bufs=3) as opool,
            # PSUM: all tiles are [128, 1024] f32 = 2 banks, so paired
            # halves evacuate in ONE DVE/ACT instruction (half the
            # per-instruction overhead).  pattn16 and prep have disjoint
            # lifetimes and share one pool (tag rotation): 1*2 + 3*2 = 8.
            tc.tile_pool(name="papp", bufs=1, space="PSUM") as pa_pool,
            tc.tile_pool(name="prec", bufs=3, space="PSUM") as pr_pool,
        ):
            wconst = cpool.tile([128, WCONST_COLS], F16)
            wbias = cpool.tile([128, WB_COLS], F32)
            scratch = cpool.tile([128, 8], F32)
            # split input DMAs across queues; touch each piece with a tiny
            # DVE copy so later consumers inherit the DVE clock and need no
            # DMA waits of their own (per-instruction wait encodings are
            # tiny).
            nc.sync.dma_start(wconst[0:64, :], wc[0:64, :])
            nc.vector.tensor_copy(scratch[0:1, 0:1], wconst[0:1, 0:1])
            nc.sync.dma_start(wconst[64:128, :], wc[64:128, :])
            nc.vector.tensor_copy(scratch[0:1, 1:2], wconst[64:65, 0:1])
            nc.sync.dma_start(wbias[0:64, :], wb[0:64, :])
            nc.vector.tensor_copy(scratch[0:1, 2:3], wbias[0:1, 0:1])
            nc.sync.dma_start(wbias[64:128, :], wb[64:128, :])
            nc.vector.tensor_copy(scratch[0:1, 3:4], wbias[64:65, 0:1])

            lhs_m1 = [wconst[32 * q:32 * q + 32, WCOMB2_C] for q in range(4)]
            lhs_wa16 = wconst[:, WA16_C]
            lhs_ws2 = wconst[:, WS2_C]
            bias_rec = wbias[:, BCOMB2_C]
            bias_bs = wbias[:, BS2_C]
            ba_vec = wbias[:, BA_C]
            zero1 = wbias[:, ZERO_C]

            for t in range(NT):
                win8 = iopool.tile([128, 1024], F16, tag="win8")
                r0 = t * 128
                nc.sync.dma_start(win8[0:64, :], xw[r0:r0 + 64, :])
                nc.vector.tensor_copy(scratch[0:1, 4:5], win8[0:1, 0:1])
                nc.sync.dma_start(win8[64:128, :], xw[r0 + 64:r0 + 128, :])
                nc.vector.tensor_copy(scratch[0:1, 5:6], win8[64:65, 0:1])

                updwin = upool.tile([128, 1024], F16, tag="updwin")

                # attn pre-act for BOTH halves into one 2-bank psum tile;
                # replicated x16 across partitions inside the matmul (relu
                # commutes with replication)
                pattn16 = pa_pool.tile([128, 1024], F32, tag="pp")
                for h in range(2):
                    nc.tensor.matmul(
                        pattn16[:, h * FH:(h + 1) * FH], lhs_wa16,
                        win8[:, h * FH:(h + 1) * FH],
                        start=True, stop=True, tile_position=(0, 0),
                    )
                attn16 = apool.tile([128, 1024], F16, tag="attn16")
                nc.vector.scalar_tensor_tensor(
                    attn16[:, :], pattn16[:, :], ba_vec,
                    zero1.broadcast_to((128, 1024)),
                    AluOpType.add, AluOpType.max,
                )

                # m1: per h, 4 row-tiled matmuls back-to-back (4-way
                # concurrent in the PE array); q pairs (0,1) and (2,3)
                # write the two bank-halves of one [128,1024] psum tile.
                precs = {}
                for h in range(2):
                    for qq in range(2):
                        prec = pr_pool.tile([128, 1024], F32, tag="prec")
                        precs[(h, qq)] = prec
                        for j in range(2):
                            q = 2 * qq + j
                            nc.tensor.matmul(
                                prec[:, j * FH:(j + 1) * FH], lhs_m1[q],
                                win8[32 * q:32 * q + 32,
                                     h * FH:(h + 1) * FH],
                                start=True, stop=True,
                                tile_position=(32 * q, 0),
                            )
                recs = {}
                for i, (h, qq) in enumerate([(0, 0), (0, 1), (1, 0), (1, 1)]):
                    rec = recpool.tile([128, 1024], F16, tag="rec")
                    recs[(h, qq)] = rec
                    if i == 0:
                        # balance: one of the four rec evacs on DVE
                        nc.vector.scalar_tensor_tensor(
                            rec[:, :], precs[(h, qq)][:, :], bias_rec,
                            zero1.broadcast_to((128, 1024)),
                            AluOpType.add, AluOpType.max,
                        )
                    else:
                        nc.scalar.activation(
                            rec[:, :], precs[(h, qq)][:, :], RELU,
                            bias=bias_rec,
                        )

                prep = pa_pool.tile([128, 1024], F32, tag="pp")
                for h in range(2):
                    for q in range(4):
                        nc.tensor.matmul(
                            prep[32 * q:32 * q + 32, h * FH:(h + 1) * FH],
                            lhs_ws2,
                            recs[(h, q // 2)][:, (q % 2) * FH:
                                              (q % 2) * FH + FH],
                            start=True, stop=True, tile_position=(0, 32 * q),
                        )

                # updwin = (prep + bs) * attn16, both halves at once
                nc.vector.scalar_tensor_tensor(
                    updwin[:, :], prep[:, :], bias_bs, attn16[:, :],
                    AluOpType.add, AluOpType.mult,
                )

                outwin = opool.tile([128, 1024], F16, tag="outwin")
                nc.gpsimd.tensor_tensor(
                    outwin[:, :], updwin[:, :], win8[:, :], AluOpType.add
                )
                nc.sync.dma_start(yw[r0:r0 + 128, :], outwin[:, :])

    if not nc.is_finalized():
        nc.finalize()
    return nc


_NC_CACHE = None


def _get_nc():
    global _NC_CACHE
    if _NC_CACHE is None:
        _NC_CACHE = _build_nc()
    return _NC_CACHE


def _host_fwd(x):
    """x [16,1,1024,1024] f32 -> per-core win8 f16 [8][2048, 1024].

    win8[tile, 16g + 4r + c, 256*ilo + jw] = x[128*tile + 16g + 4*ilo + r,
                                               4*jw + c]
    """
    X = np.asarray(x, np.float32).reshape(B * H, W)
    T = X.reshape(128, 8, 4, 4, 256, 4)          # [t, g, ilo, r, jw, c]
    Wn = T.transpose(0, 1, 3, 5, 2, 4)           # [t, g, r, c, ilo, jw]
    win = np.ascontiguousarray(Wn).astype(np.float16)
    return win.reshape(N_CORES, ROWS, W)


def _host_inv(yw):
    """yw [8][2048, 1024] f16 (window layout) -> y [16,1,1024,1024] f32."""
    wf = yw.reshape(128, 8, 4, 4, 4, 256).astype(np.float32)
    Y = wf.transpose(0, 1, 4, 2, 5, 3).reshape(B * H, W)
    return np.ascontiguousarray(Y).reshape(B, 1, H, W)


def _in_maps(inputs):
    wconst, wdr8, wbias = _build_wconst(
        np.asarray(inputs["Wa"], np.float32), np.asarray(inputs["ba"], np.float32),
        np.asarray(inputs["We"], np.float32), np.asarray(inputs["be"], np.float32),
        np.asarray(inputs["Wr"], np.float32), np.asarray(inputs["br"], np.float32),
        np.asarray(inputs["Ws"], np.float32), np.asarray(inputs["bs"], np.float32),
    )
    win = _host_fwd(inputs["x"])
    return [{"xw": win[core], "wc": wconst, "wb": wbias}
            for core in range(N_CORES)]


def kernel(x, Wa, ba, We, be, Wr, br, Ws, bs, **_ignored):
    in_maps = _in_maps(dict(x=x, Wa=Wa, ba=ba, We=We, be=be, Wr=Wr, br=br,
                            Ws=Ws, bs=bs))
    nc = _get_nc()
    res = run_bass_kernel_spmd(nc, in_maps, list(range(N_CORES)))
    yw = np.stack([np.asarray(res.results[i]["yw"]) for i in range(N_CORES)])
    return _host_inv(yw)
